# revision 1
# baseline (speedup 1.0000x reference)
"""Trainium2 Bass kernel for the 2-layer grid-GCN + linear head.

Math: the GCN aggregation over the fixed graph is a linear operator on
the node axis: out = A @ h per batch column, where
A[j, i] = sum_{edges (i->j)} dinv[i]*dinv[j].  For the 26x26 grid with
row-major node order A is banded (|i-j| <= 26), so with 128-row node
tiles it is block-tridiagonal.  The whole network becomes

    h1 = relu(B1 @ xT + b1)      B1 = w1 * A   (bf16 stationaries)
    h2 = relu(B2 @ h1 + b2)      B2 = w2 * A
    y  = relu(linw.T @ h2 + lin_b)

computed per 512-wide batch-column chunk on the tensor engine, with
ScalarE (conv1 + head) and VectorE (conv2) draining PSUM through the
relu + bf16 cast.  Batch is sharded across the 8 NeuronCores (pure data
parallel); x is transposed and cast to bf16 on the host so every DMA is
a clean 2D pattern.
"""

import sys

if "/opt/trn_rl_repo" not in sys.path:
    sys.path.insert(0, "/opt/trn_rl_repo")

import numpy as np
import ml_dtypes

N_CORES = 8
N = 676           # nodes (26x26 grid)
B_TOTAL = 65536
COLS = B_TOTAL // N_CORES      # batch columns per core
CHUNK = 512                    # matmul free dim / PSUM bank
GROUP = 2048                   # DMA column-group
N_CHUNKS = COLS // CHUNK
N_GROUPS = COLS // GROUP
N_TILES = (N + 127) // 128     # 6 node tiles
P = [min(128, N - 128 * t) for t in range(N_TILES)]   # [128]*5 + [36]
OFF = [128 * t for t in range(N_TILES)]

bf16 = ml_dtypes.bfloat16

TRACE = False            # test.py flips this to profile
LAST_RESULT = None       # BassKernelResults stash when TRACE


def _neighbors(m):
    return [k for k in (m - 1, m, m + 1) if 0 <= k < N_TILES]


_BOFF = {}
_W = 0
for _m in range(N_TILES):
    for _k in _neighbors(_m):
        _BOFF[(_m, _k)] = _W
        _W += P[_m]


DIAG_OFF = [sum(P[:m]) for m in range(N_TILES)]
DIAG_W = sum(P)
COR_W = 4 * 128 + 128 + P[-1]    # 4 packed pairs + lo(4) + full-K hi(5)
LO4_OFF = 4 * 128
HI5_OFF = 5 * 128


def _pack_blocks(Bmat):
    """Pack lhsT blocks of the block-tridiagonal operator.

    diag [128, 676]: block m = Bmat[tile m, tile m] at DIAG_OFF[m].
    cor  [128, 676]:
      pair i (i=0..3) at cols 128*i, shared column range:
        rows  0:32  -> lo(i):  first-32-rows window of tile i+1 -> out-tile i
        rows 64:128 -> hi(i+1): last-64-rows window of tile i  -> out-tile i+1
      (the two run concurrently in disjoint PE row groups)
      lo(4) at cols LO4_OFF (rows 0:32 of tile 5 -> out-tile 4)
      hi(5) at cols HI5_OFF: FULL-K block of tile 4 -> out-tile 5
        (K=64/base-64 into an M=36 psum hard-faults the HW - probed)
    """
    diag = np.zeros((128, DIAG_W), dtype=bf16)
    for m in range(N_TILES):
        blk = Bmat[OFF[m]:OFF[m] + P[m], OFF[m]:OFF[m] + P[m]]
        diag[: P[m], DIAG_OFF[m]:DIAG_OFF[m] + P[m]] = blk.astype(bf16)
    cor = np.zeros((128, COR_W), dtype=bf16)
    for i in range(4):
        c = 128 * i
        cor[0:32, c:c + 128] = Bmat[OFF[i + 1]:OFF[i + 1] + 32,
                                    OFF[i]:OFF[i] + 128].astype(bf16)
        cor[64:128, c:c + 128] = Bmat[OFF[i + 1] - 64:OFF[i + 1],
                                      OFF[i + 1]:OFF[i + 1] + 128].astype(bf16)
    cor[0:32, LO4_OFF:LO4_OFF + 128] = Bmat[OFF[5]:OFF[5] + 32,
                                            OFF[4]:OFF[4] + 128].astype(bf16)
    cor[0:128, HI5_OFF:HI5_OFF + P[5]] = Bmat[OFF[4]:OFF[4] + 128,
                                              OFF[5]:OFF[5] + P[5]].astype(bf16)
    return diag, cor


_PROGRAM_CACHE = {}


def _build_program(b1f, b2f, linbf):
    key = (b1f, b2f, linbf)
    if key in _PROGRAM_CACHE:
        return _PROGRAM_CACHE[key]

    import concourse.mybir as mybir
    import concourse.tile as tile
    from concourse import bacc

    nc = bacc.Bacc(None, target_bir_lowering=False)
    dt = mybir.dt

    xt_d = nc.dram_tensor("xt", (N, COLS), dt.bfloat16, kind="ExternalInput")
    wd1_d = nc.dram_tensor("wd1", (128, DIAG_W), dt.bfloat16, kind="ExternalInput")
    wd2_d = nc.dram_tensor("wd2", (128, DIAG_W), dt.bfloat16, kind="ExternalInput")
    wr1_d = nc.dram_tensor("wr1", (128, COR_W), dt.bfloat16, kind="ExternalInput")
    wr2_d = nc.dram_tensor("wr2", (128, COR_W), dt.bfloat16, kind="ExternalInput")
    wlin_d = nc.dram_tensor("wlin", (128, N_TILES), dt.bfloat16, kind="ExternalInput")
    y_d = nc.dram_tensor("y", (1, COLS), dt.float32, kind="ExternalOutput")

    with tile.TileContext(nc) as tc:
        with (
            tc.tile_pool(name="weights", bufs=1) as wpool,
            tc.tile_pool(name="xin", bufs=2) as xpool,
            tc.tile_pool(name="acts", bufs=2) as hpool,
            tc.tile_pool(name="yout", bufs=1) as ypool,
            tc.tile_pool(name="ps1", bufs=3, space="PSUM") as ps1pool,
            tc.tile_pool(name="ps2", bufs=3, space="PSUM") as ps2pool,
            tc.tile_pool(name="psl", bufs=2, space="PSUM") as pslpool,
        ):
            # x chunk 0 first so compute starts ASAP, then weights, then rest
            xt_tiles = [[None] * N_GROUPS for _ in range(N_TILES)]
            for t in range(N_TILES):
                xt_tiles[t][0] = xpool.tile([P[t], GROUP], dt.bfloat16,
                                            tag=f"x{t}", name=f"x{t}_0")
                nc.sync.dma_start(
                    xt_tiles[t][0][:, 0:CHUNK],
                    xt_d[OFF[t]:OFF[t] + P[t], 0:CHUNK],
                )

            wd1 = wpool.tile([128, DIAG_W], dt.bfloat16, tag="wd1")
            wd2 = wpool.tile([128, DIAG_W], dt.bfloat16, tag="wd2")
            wr1 = wpool.tile([128, COR_W], dt.bfloat16, tag="wr1")
            wr2 = wpool.tile([128, COR_W], dt.bfloat16, tag="wr2")
            wlin = wpool.tile([128, N_TILES], dt.bfloat16, tag="wlin")
            nc.sync.dma_start(wd1[:], wd1_d[:])
            nc.sync.dma_start(wd2[:], wd2_d[:])
            nc.sync.dma_start(wr1[:], wr1_d[:])
            nc.sync.dma_start(wr2[:], wr2_d[:])
            nc.sync.dma_start(wlin[:], wlin_d[:])

            for t in range(N_TILES):
                nc.sync.dma_start(
                    xt_tiles[t][0][:, CHUNK:GROUP],
                    xt_d[OFF[t]:OFF[t] + P[t], CHUNK:GROUP],
                )

            y_sb = ypool.tile([1, COLS], dt.float32, tag="y")
            relu = mybir.ActivationFunctionType.Relu

            for c in range(N_CHUNKS):
                g = c // (GROUP // CHUNK)
                if c % (GROUP // CHUNK) == 0 and g > 0:
                    for t in range(N_TILES):
                        xt_tiles[t][g] = xpool.tile(
                            [P[t], GROUP], dt.bfloat16, tag=f"x{t}",
                            name=f"x{t}_{g}",
                        )
                        nc.sync.dma_start(
                            xt_tiles[t][g][:],
                            xt_d[OFF[t]:OFF[t] + P[t],
                                 g * GROUP:(g + 1) * GROUP],
                        )
                cs = slice((c % (GROUP // CHUNK)) * CHUNK,
                           (c % (GROUP // CHUNK) + 1) * CHUNK)

                def emit_conv(wd, wr, rhs_of, pspool, pstag, drain):
                    """6 diag MMs + packed corner pairs (disjoint 32/64-row
                    PE groups run concurrently) + full-K m=5 corner."""
                    ps = [None] * N_TILES
                    for m in range(N_TILES):
                        ps[m] = pspool.tile([P[m], CHUNK], dt.float32,
                                            tag=pstag, name=f"{pstag}_{m}")
                        nc.tensor.matmul(
                            ps[m][:],
                            wd[: P[m], DIAG_OFF[m]:DIAG_OFF[m] + P[m]],
                            rhs_of(m),
                            start=True, stop=False,
                        )
                        if 1 <= m <= 4:
                            i = m - 1
                            nc.tensor.matmul(          # lo(i) closes psum i
                                ps[i][:],
                                wr[0:32, 128 * i:128 * i + 128],
                                rhs_of(m)[0:32, :],
                                start=False, stop=True,
                            )
                            nc.tensor.matmul(          # hi(m), rows 64:128
                                ps[m][:],
                                wr[64:128, 128 * i:128 * i + 128],
                                rhs_of(i)[64:128, :],
                                start=False, stop=False,
                            )
                            drain(i, ps[i])
                        elif m == 5:
                            nc.tensor.matmul(          # lo(4) closes psum 4
                                ps[4][:],
                                wr[0:32, LO4_OFF:LO4_OFF + 128],
                                rhs_of(5)[0:32, :],
                                start=False, stop=True,
                            )
                            nc.tensor.matmul(          # hi(5) full-K
                                ps[5][:],
                                wr[0:128, HI5_OFF:HI5_OFF + P[5]],
                                rhs_of(4),
                                start=False, stop=True,
                            )
                            drain(4, ps[4])
                            drain(5, ps[5])

                # ---- conv1: h1 = relu(B1 @ xT + b1) ----
                h1 = [None] * N_TILES

                def drain1(m, ps):
                    h = hpool.tile([P[m], CHUNK], dt.bfloat16,
                                   tag=f"h1_{m}", name=f"h1_{m}")
                    nc.scalar.activation(h[:], ps[:], relu, bias=b1f)
                    h1[m] = h

                emit_conv(wd1, wr1, lambda k: xt_tiles[k][g][:, cs],
                          ps1pool, "ps1", drain1)

                # ---- conv2: h2 = relu(B2 @ h1 + b2) ----
                h2 = [None] * N_TILES

                def drain2(m, ps):
                    h = hpool.tile([P[m], CHUNK], dt.bfloat16,
                                   tag=f"h2_{m}", name=f"h2_{m}")
                    if b2f == 0.0:
                        nc.vector.tensor_scalar_max(h[:], ps[:], 0.0)
                    else:
                        nc.vector.tensor_scalar(
                            h[:], ps[:], b2f, 0.0,
                            mybir.AluOpType.add, mybir.AluOpType.max,
                        )
                    h2[m] = h

                emit_conv(wd2, wr2, lambda k: h1[k][:],
                          ps2pool, "ps2", drain2)

                # ---- linear head: y = relu(linw.T @ h2 + lin_b) ----
                psl = pslpool.tile([1, CHUNK], dt.float32, tag="psl",
                                   name="psl")
                for k in range(N_TILES):
                    nc.tensor.matmul(
                        psl[:],
                        wlin[: P[k], k:k + 1],
                        h2[k][:],
                        start=(k == 0),
                        stop=(k == N_TILES - 1),
                    )
                nc.scalar.activation(
                    y_sb[0:1, c * CHUNK:(c + 1) * CHUNK], psl[:], relu,
                    bias=linbf,
                )

            nc.sync.dma_start(y_d[:], y_sb[:])

    nc.compile()
    _PROGRAM_CACHE[key] = nc
    return nc


def kernel(x, w1, b1, w2, b2, lin_w, lin_b, edge_src, edge_dst):
    global LAST_RESULT
    from concourse import bass_utils

    x = np.asarray(x)
    # Build the dense normalized aggregation operator from the edge lists.
    deg = np.zeros(N, np.float64)
    np.add.at(deg, np.asarray(edge_dst), 1.0)
    dinv = 1.0 / np.sqrt(deg)
    normv = dinv[np.asarray(edge_src)] * dinv[np.asarray(edge_dst)]
    A = np.zeros((N, N), np.float64)
    np.add.at(A, (np.asarray(edge_dst), np.asarray(edge_src)), normv)

    w1f = float(np.asarray(w1).reshape(-1)[0])
    w2f = float(np.asarray(w2).reshape(-1)[0])
    b1f = float(np.asarray(b1).reshape(-1)[0])
    b2f = float(np.asarray(b2).reshape(-1)[0])
    linbf = float(np.asarray(lin_b).reshape(-1)[0])

    wd1_np, wr1_np = _pack_blocks((w1f * A).astype(np.float32))
    wd2_np, wr2_np = _pack_blocks((w2f * A).astype(np.float32))
    wlin_np = np.zeros((128, N_TILES), dtype=bf16)
    lw = np.asarray(lin_w).reshape(-1)
    for t in range(N_TILES):
        wlin_np[: P[t], t] = lw[OFF[t]:OFF[t] + P[t]].astype(bf16)

    nc = _build_program(b1f, b2f, linbf)

    # host-side: transpose, cast, shard along batch
    xt = np.ascontiguousarray(x.T).astype(bf16)        # [676, 65536]
    in_maps = []
    for c in range(N_CORES):
        in_maps.append({
            "xt": np.ascontiguousarray(xt[:, c * COLS:(c + 1) * COLS]),
            "wd1": wd1_np,
            "wd2": wd2_np,
            "wr1": wr1_np,
            "wr2": wr2_np,
            "wlin": wlin_np,
        })

    res = bass_utils.run_bass_kernel_spmd(
        nc, in_maps, list(range(N_CORES)), trace=TRACE
    )
    if TRACE:
        LAST_RESULT = res
    out = np.concatenate([res.results[c]["y"].reshape(-1) for c in range(N_CORES)])
    return out.reshape(B_TOTAL, 1).astype(np.float32)

